# revision 29
# baseline (speedup 1.0000x reference)
"""Trainium2 distributed kernel for nn_AutoCorrelationLayer (FourierBlock).

Reference math:
    q   = queries @ Wq.T + bq                  (B, L, E)
    xf  = rfft(q, axis=1)[:, :M, :]            keep 32 low modes
    y_m = xf_m @ (W1r_m + i W1i_m)             per-mode ExE complex mix
    x   = irfft(pad(y), n=L, axis=1)
    out = x @ Wo.T + bo

Only M=32 of 1025 frequency bins survive, so the FFTs collapse to small
DFT matmuls, and Wq / Wo can be folded into the per-mode weights on the
host:  W'_m = Wq.T @ (W1r_m + i W1i_m) @ Wo.T.  Device pipeline:

  A: qf_m  = DFT_lo(queries)                  batch-parallel (4 batches/core)
  -- AllToAll: batch-shard -> mode-shard
  B: y_m   = qf_m @ W'_m                      mode-parallel  (4 modes/core)
  -- AllToAll: mode-shard -> batch-shard
  C: out   = iDFT_lo(y)                       batch-parallel

Raw bass (manual semaphores): walrus's DIRECT2D DMA template accepts at
most one sync wait, so all DMA ordering is done with engine-level
wait_ge instructions and program order on the two HWDGE rings (SP=input
streams, ACT=weights/evictions/output).  All matmuls run in bf16 with
f32 PSUM accumulation.
"""

import sys
from contextlib import ExitStack

import numpy as np

sys.path.insert(0, "/opt/trn_rl_repo")

import concourse.bass as bass  # noqa: E402
import concourse.mybir as mybir  # noqa: E402
from concourse.bass_utils import run_bass_kernel_spmd  # noqa: E402

import ml_dtypes  # noqa: E402

BF16 = ml_dtypes.bfloat16

B, L, E, MODES = 32, 2048, 512, 32
NCORES = 8
BL = B // NCORES          # local batches per core (4)
ML = MODES // NCORES      # local modes per core (4)
NCH = E // 128            # 128-partition chunks of E (4)
KT = L // 128             # k-tiles along L (16)
GRP = 2 * ML              # cols per mode-group in DFT output (4 cos + 4 sin)

_nc_cache = None


def build_nc():
    f32 = mybir.dt.float32
    bf16 = mybir.dt.bfloat16

    nc = bass.Bass()

    q_ext = nc.declare_dram_parameter("q", [BL, L, E], bf16, isOutput=False)
    ft_ext = nc.declare_dram_parameter("ft", [128, KT * 64], bf16, isOutput=False)
    w_ext = nc.declare_dram_parameter("w", [ML, 128, 2 * NCH * E], bf16, isOutput=False)
    g_ext = nc.declare_dram_parameter("g", [128, L], bf16, isOutput=False)
    mb_ext = nc.declare_dram_parameter("mb", [B, E], f32, isOutput=False)
    out_ext = nc.declare_dram_parameter("out", [BL, L, E], bf16, isOutput=True)

    # A2A bounces. a1: [dest j][b 4][i 512][col 8]; a2: [dest j][b 4][tr 8][p 512]
    a1_in = nc.dram_tensor("a1_in", [NCORES, BL * E * GRP], bf16)
    a1_out = nc.dram_tensor("a1_out", [NCORES, BL * E * GRP], bf16)
    a2_in = nc.dram_tensor("a2_in", [NCORES, BL * GRP * E], bf16)
    a2_out = nc.dram_tensor("a2_out", [NCORES, BL * GRP * E], bf16)
    wa_in = nc.dram_tensor("wa_in", [NCORES, 16], bf16)
    wa_out = nc.dram_tensor("wa_out", [NCORES, 16], bf16)
    rg = [list(range(NCORES))]

    with ExitStack() as ctx:
        ft_sb = ctx.enter_context(nc.sbuf_tensor([128, KT * 64], bf16))
        w_sb = ctx.enter_context(nc.sbuf_tensor([128, ML * 2 * NCH * E], bf16))
        g_sb = ctx.enter_context(nc.sbuf_tensor([128, L], bf16))
        mb_sb = ctx.enter_context(nc.sbuf_tensor([B, E], f32))
        qk_sb = ctx.enter_context(nc.sbuf_tensor([128, 2 * KT * E], bf16))
        qa_sb = ctx.enter_context(nc.sbuf_tensor([128, BL * NCH * 64], bf16))
        qm_sb = ctx.enter_context(nc.sbuf_tensor([128, NCH * B * GRP], bf16))
        nq_sb = ctx.enter_context(nc.sbuf_tensor([128, NCH * B * GRP], bf16))
        ys_sb = ctx.enter_context(nc.sbuf_tensor([128, 2 * E], bf16))
        yst_sb = ctx.enter_context(nc.sbuf_tensor([128, 2 * E], bf16))
        ob_sb = ctx.enter_context(nc.sbuf_tensor([128, 2 * 16 * E], bf16))
        ps = ctx.enter_context(nc.psum_tensor([128, 4096], f32))
        (sFt, sMb, sW, sG, sMA, sEA, sS1, sCC, sQM, sNG, sMB, sAD, sEB, sS2,
         sMC, sECa, sECv) = (
            ctx.enter_context(nc.semaphore(n))
            for n in ("sFt", "sMb", "sW", "sG", "sMA", "sEA", "sS1", "sCC",
                      "sQM", "sNG", "sMB", "sAD", "sEB", "sS2", "sMC", "sECa",
                      "sECv")
        )
        sQh = [ctx.enter_context(nc.semaphore(f"sQ{i}")) for i in range(4)]
        sYs = [ctx.enter_context(nc.semaphore(f"sY{i}")) for i in range(2)]
        sOb = [ctx.enter_context(nc.semaphore(f"sO{i}")) for i in range(2)]
        sOg = [ctx.enter_context(nc.semaphore(f"sOg{i}")) for i in range(2)]
        block = ctx.enter_context(nc.Block())
        # views
        def qk_v(b, k):
            return qk_sb[:, ((b % 2) * KT + k) * E : ((b % 2) * KT + k + 1) * E]

        def ft_v(k):
            return ft_sb[:, 64 * k : 64 * (k + 1)]

        def w_v(t, j, ch):
            o = ((t * 2 + j) * NCH + ch) * E
            return w_sb[:, o : o + E]

        def psA_v(b, ch):
            bank = (b % 2) * 4 + ch
            return ps[:, 512 * bank : 512 * bank + 64]

        def psB_v(t, ri):
            x = 2 * (t % 2) + ri
            bank = 4 * (t // 2) + x
            return ps[32 * x : 32 * (x + 1), 512 * bank : 512 * (bank + 1)]

        def psC_v(idx):
            bank = idx % 8
            return ps[:, 512 * bank : 512 * (bank + 1)]

        qa_r = qa_sb.rearrange(
            "p (j b ch u) -> p j b ch u", j=NCORES, b=BL, ch=NCH, u=GRP
        )

        def qa_v(b, ch):
            return qa_r[:, :, b, ch, :]  # (128, 8, 8) strided

        qm_r = qm_sb.rearrange(
            "p (jb ch u) -> p ch u jb", jb=B, ch=NCH, u=GRP
        )
        nq_r = nq_sb.rearrange(
            "p (jb ch u) -> p ch u jb", jb=B, ch=NCH, u=GRP
        )

        def ys_v(t, ri):
            x = 2 * (t % 2) + ri
            return ys_sb[32 * x : 32 * (x + 1), (t // 2) * E : (t // 2 + 1) * E]

        def yst_v(b):
            return yst_sb[:, (b % 2) * E : ((b % 2) + 1) * E]

        def ob_v(idx):
            o = ((idx // 16) % 2) * 16 + idx % 16
            return ob_sb[:, o * E : (o + 1) * E]

        def ob_b(bb):
            o = (bb % 2) * 16
            return ob_sb[:, o * E : (o + 16) * E]

        def cdec(idx):
            return idx // 16, idx % 16  # b, l-chunk

        EV_ENG = ["a" if i % 8 < 5 else "v" for i in range(BL * 16)]

        def ev_count(eng, upto):
            return sum(1 for i in range(upto + 1) if EV_ENG[i] == eng)


        # ---------------- SP ring: input streams ----------------
        @block.sync
        def _(sync):
            for b in range(BL):
                if b >= 2:
                    sync.wait_ge(sMA, 4 * (b - 1))  # batch b-2 fully consumed
                for h in range(2):
                    sync.dma_start(
                        out=qk_sb.rearrange("p (s k e) -> p s k e", s=4, k=KT // 2)[
                            :, (b % 2) * 2 + h
                        ],
                        in_=q_ext[b].rearrange("(k p) e -> p k e", p=128)[
                            :, 8 * h : 8 * (h + 1)
                        ],
                    ).then_inc(sQh[(b % 2) * 2 + h], 16)
            # qm load (after collective 1)
            sync.wait_ge(sCC, 2)
            sync.dma_start(
                out=qm_sb.rearrange("p (j f) -> p j f", j=NCORES),
                in_=a1_out.rearrange("j (p f) -> p j f", p=128),
            ).then_inc(sQM, 16)
            # yst loads (after collective 2) interleaved with out stores
            sync.wait_ge(sCC, 3)

            def out_dmas(bb):
                na = ev_count("a", 16 * bb + 7)
                nv = ev_count("v", 16 * bb + 7)
                if na:
                    sync.wait_ge(sECa, na)
                if nv:
                    sync.wait_ge(sECv, nv)
                sync.dma_start(
                    out=out_ext[bb, 0:1024, :].rearrange(
                        "(lch p) e -> p lch e", p=128
                    ),
                    in_=ob_b(bb)[:, 0 : 8 * E].rearrange(
                        "p (lch e) -> p lch e", e=E
                    ),
                ).then_inc(sOb[bb % 2], 16)

            for b in range(BL):
                if b >= 2:
                    sync.wait_ge(sMC, 16 * (b - 1))  # batch b-2 C-matmuls done
                for half in range(2):
                    sync.dma_start(
                        out=yst_v(b)[64 * half : 64 * (half + 1), :],
                        in_=a2_out.rearrange(
                            "j (x bl pp p) -> bl j x pp p", x=4, bl=BL, pp=2, p=E
                        )[b],
                    ).then_inc(sYs[b % 2], 16)
                if b >= 1:
                    out_dmas(b - 1)
            out_dmas(BL - 1)

        # ---------------- PE: all matmuls ----------------
        @block.tensor
        def _(pe):
            pe.wait_ge(sFt, 16)  # ft loaded
            for b in range(BL):
                if b >= 2:
                    pe.wait_ge(sEA, 4 * (b - 1))  # psum bank set evicted
                for k in range(KT):
                    if k % 8 == 0:
                        pe.wait_ge(sQh[(b % 2) * 2 + k // 8], 16 * (b // 2 + 1))
                    for ch in range(NCH):
                        mm = pe.matmul(
                            psA_v(b, ch),
                            qk_v(b, k)[:, 128 * ch : 128 * (ch + 1)],
                            ft_v(k),
                            start=(k == 0),
                            stop=(k == KT - 1),
                        )
                        if k == KT - 1:
                            mm.then_inc(sMA, 1)
            # stage B
            pe.wait_ge(sW, 64)
            pe.wait_ge(sG, 16)
            pe.wait_ge(sQM, 16)
            pe.wait_ge(sNG, 1)
            for ps_i_ in range(2):
                for ch in range(NCH):
                    for tl in range(2):
                        t = 2 * ps_i_ + tl
                        lhs_r = qm_r[:, ch, t, :]
                        lhs_i = qm_r[:, ch, ML + t, :]
                        lhs_ni = nq_r[:, ch, ML + t, :]
                        first, last = ch == 0, ch == NCH - 1
                        tp0 = (0, 32 * (2 * tl + 0))
                        tp1 = (0, 32 * (2 * tl + 1))
                        pe.matmul(psB_v(t, 0), lhs_r, w_v(t, 0, ch),
                                  start=first, stop=False, tile_position=tp0)
                        pe.matmul(psB_v(t, 1), lhs_r, w_v(t, 1, ch),
                                  start=first, stop=False, tile_position=tp1)
                        m3 = pe.matmul(psB_v(t, 0), lhs_ni, w_v(t, 1, ch),
                                       start=False, stop=last, tile_position=tp0)
                        m4 = pe.matmul(psB_v(t, 1), lhs_i, w_v(t, 0, ch),
                                       start=False, stop=last, tile_position=tp1)
                        if last:
                            m3.then_inc(sMB, 1)
                            m4.then_inc(sMB, 1)
            # stage C
            for b in range(BL):
                pe.wait_ge(sYs[b % 2], 32 * (b // 2 + 1))
                for lch in range(0, 16, 2):
                    idx = b * 16 + lch
                    if idx >= 8:
                        na = ev_count("a", idx - 7)
                        nv = ev_count("v", idx - 7)
                        if na:
                            pe.wait_ge(sECa, na)
                        if nv:
                            pe.wait_ge(sECv, nv)
                    pe.matmul(
                        psC_v(idx),
                        g_sb[0:64, 128 * lch : 128 * (lch + 1)],
                        yst_v(b)[0:64, :],
                        start=True,
                        stop=True,
                        tile_position=(0, 0),
                    ).then_inc(sMC, 1)
                    pe.matmul(
                        psC_v(idx + 1),
                        g_sb[64:128, 128 * (lch + 1) : 128 * (lch + 2)],
                        yst_v(b)[64:128, :],
                        start=True,
                        stop=True,
                        tile_position=(64, 0),
                    ).then_inc(sMC, 1)

        # ------------- ACT ring: consts, evictions, staging, output -------------
        @block.scalar
        def _(act):
            act.dma_start(out=ft_sb[:], in_=ft_ext[:]).then_inc(sFt, 16)
            act.dma_start(out=mb_sb[:], in_=mb_ext[:]).then_inc(sMb, 16)
            act.dma_start(out=g_sb[:], in_=g_ext[:]).then_inc(sG, 16)
            # stage A evictions (f32 -> bf16)
            for b in range(BL):
                for ch in range(NCH):
                    act.wait_ge(sMA, 4 * b + ch + 1)
                    act.copy(
                        out=qa_v(b, ch),
                        in_=psA_v(b, ch).rearrange("p (j u) -> p j u", j=NCORES),
                    ).then_inc(sEA, 1)
            # staging 1
            act.wait_ge(sEA, 16)
            act.dma_start(
                out=a1_in.rearrange("j (p f) -> p j f", p=128),
                in_=qa_sb.rearrange("p (j f) -> p j f", j=NCORES),
            ).then_inc(sS1, 16)
            # w loads drain during collective 1
            for t in range(ML):
                act.dma_start(
                    out=w_sb[:, t * 2 * NCH * E : (t + 1) * 2 * NCH * E],
                    in_=w_ext[t],
                ).then_inc(sW, 16)
            # stage B evictions
            for t in range(ML):
                for ri in range(2):
                    if t == 0 and ri == 0:
                        act.wait_ge(sAD, 1)
                    else:
                        act.wait_ge(sMB, 4 * (t // 2) + 2 * (t % 2) + ri + 1)
                    act.copy(out=ys_v(t, ri), in_=psB_v(t, ri)).then_inc(sEB, 1)
            # staging 2 (per col-group as its evictions finish)
            for x in range(4):
                act.wait_ge(sEB, 5 + x)
                act.dma_start(
                    out=a2_in.rearrange("j (x blc) -> x j blc", x=4, blc=BL * 2 * E)[x],
                    in_=ys_sb[32 * x : 32 * (x + 1), :],
                ).then_inc(sS2, 16)
            # stage C evictions (ACT share)
            for idx in range(BL * 16):
                if EV_ENG[idx] != "a":
                    continue
                act.wait_ge(sMC, idx + 1)
                bb = idx // 16
                if bb >= 2 and idx % 16 == 0:
                    act.wait_ge(sOb[bb % 2], 16 * ((bb - 2) // 2 + 1))
                    act.wait_ge(sOg[bb % 2], 16 * ((bb - 2) // 2 + 1))
                act.copy(out=ob_v(idx), in_=psC_v(idx)).then_inc(sECa, 1)

        # ------------- DVE: negation, bias add, half the C evictions -------------
        @block.vector
        def _(dve):
            dve.wait_ge(sQM, 16)
            dve.tensor_scalar_mul(nq_sb[:], qm_sb[:], -1.0).then_inc(sNG, 1)
            dve.wait_ge(sMb, 16)  # mb loaded
            dve.wait_ge(sMB, 1)   # t=0 yr chain done
            dve.tensor_add(psB_v(0, 0), psB_v(0, 0), mb_sb[:]).then_inc(sAD, 1)
            first_v = min(i for i in range(16) if EV_ENG[i] == "v")
            for idx in range(BL * 16):
                if EV_ENG[idx] != "v":
                    continue
                dve.wait_ge(sMC, idx + 1)
                bb = idx // 16
                if bb >= 2 and idx % 16 == first_v:
                    dve.wait_ge(sOb[bb % 2], 16 * ((bb - 2) // 2 + 1))
                    dve.wait_ge(sOg[bb % 2], 16 * ((bb - 2) // 2 + 1))
                dve.tensor_copy(ob_v(idx), psC_v(idx)).then_inc(sECv, 1)

        # ---------------- GPSIMD: collectives ----------------
        @block.gpsimd
        def _(gp):
            gp.collective_compute(
                "AllToAll",
                mybir.AluOpType.bypass,
                replica_groups=rg,
                ins=[wa_in[:]],
                outs=[wa_out[:]],
            ).then_inc(sCC, 1)
            gp.wait_ge(sS1, 16)
            gp.collective_compute(
                "AllToAll",
                mybir.AluOpType.bypass,
                replica_groups=rg,
                ins=[a1_in[:]],
                outs=[a1_out[:]],
            ).then_inc(sCC, 1)
            gp.wait_ge(sS2, 64)
            gp.collective_compute(
                "AllToAll",
                mybir.AluOpType.bypass,
                replica_groups=rg,
                ins=[a2_in[:]],
                outs=[a2_out[:]],
            ).then_inc(sCC, 1)
            for bb in range(BL):
                gp.wait_ge(sECa, ev_count("a", 16 * bb + 15))
                gp.wait_ge(sECv, ev_count("v", 16 * bb + 15))
                gp.dma_start(
                    out=out_ext[bb, 1024:2048, :].rearrange(
                        "(lch p) e -> p lch e", p=128
                    ),
                    in_=ob_b(bb)[:, 8 * E : 16 * E].rearrange(
                        "p (lch e) -> p lch e", e=E
                    ),
                ).then_inc(sOg[bb % 2], 16)

    return nc


def _host_prep(queries, Wq, bq, W1r, W1i, Wo, bo):
    """Fold Wq/Wo into per-mode weights, build DFT matrices, shard per core."""
    l = np.arange(L)
    m = np.arange(MODES)
    ang = 2.0 * np.pi * np.outer(m, l) / L          # (M, L)
    cos_ml = np.cos(ang)
    sin_ml = np.sin(ang)

    # DFT moving tiles, packed [p_in_tile, k*64 + c]; c: group g -> [cos, -sin]
    ft = np.empty((L, 64), np.float32)
    for g in range(NCORES):
        ft[:, GRP * g : GRP * g + ML] = cos_ml[4 * g : 4 * g + ML].T
        ft[:, GRP * g + ML : GRP * (g + 1)] = -sin_ml[4 * g : 4 * g + ML].T
    ft = np.ascontiguousarray(
        ft.reshape(KT, 128, 64).transpose(1, 0, 2).reshape(128, KT * 64)
    )

    # Folded mode weights: W'_m = Wq.T @ (W1r_m + i W1i_m) @ Wo.T
    Wq64 = Wq.astype(np.float64)
    Wo64 = Wo.astype(np.float64)
    Wpr = np.empty((E, E, MODES), np.float32)
    Wpi = np.empty((E, E, MODES), np.float32)
    for mm in range(MODES):
        ar = Wq64.T @ W1r[:, :, mm].astype(np.float64)
        ai = Wq64.T @ W1i[:, :, mm].astype(np.float64)
        Wpr[:, :, mm] = (ar @ Wo64.T).astype(np.float32)
        Wpi[:, :, mm] = (ai @ Wo64.T).astype(np.float32)

    # Inverse DFT rows g[k = j*8 + t*2 + ri, l]
    cm = np.where(m == 0, 1.0, 2.0)
    g_mat = np.empty((64, L), np.float32)
    for r in range(64):
        c, x, pp = r // 8, (r % 8) // 2, r % 2
        tl, ri = x // 2, x % 2
        mm = 4 * c + 2 * pp + tl
        if ri == 0:
            g_mat[r] = cm[mm] * cos_ml[mm] / L
        else:
            g_mat[r] = -cm[mm] * sin_ml[mm] / L
            if mm == 0:
                g_mat[r] = 0.0  # irfft ignores Im(bin 0)

    out_bias = (
        bo.astype(np.float64)
        + bq.astype(np.float64) @ W1r[:, :, 0].astype(np.float64) @ Wo64.T
    ).astype(np.float32)

    ft_b = ft.astype(BF16)
    g_b = np.vstack([g_mat, g_mat]).astype(BF16)

    in_maps = []
    for c in range(NCORES):
        w_pack = np.empty((ML, 128, 2, NCH, E), np.float32)
        for t in range(ML):
            mm = 4 * c + t
            for ch in range(NCH):
                w_pack[t, :, 0, ch] = Wpr[128 * ch : 128 * (ch + 1), :, mm]
                w_pack[t, :, 1, ch] = Wpi[128 * ch : 128 * (ch + 1), :, mm]
        w_pack = w_pack.reshape(ML, 128, 2 * NCH * E)
        in_maps.append(
            {
                "q": np.ascontiguousarray(queries[BL * c : BL * (c + 1)]).astype(BF16),
                "ft": ft_b,
                "w": w_pack.astype(BF16),
                "g": g_b,
                "mb": np.broadcast_to(
                    L * out_bias[None, :] if c == 0 else np.zeros((1, E), np.float32),
                    (B, E),
                ).astype(np.float32),
            }
        )
    return in_maps


def kernel(queries, Wq, bq, W1r, W1i, Wo, bo, _trace=False):
    global _nc_cache
    if _nc_cache is None:
        _nc_cache = build_nc()
    nc = _nc_cache

    in_maps = _host_prep(queries, Wq, bq, W1r, W1i, Wo, bo)
    res = run_bass_kernel_spmd(nc, in_maps, core_ids=list(range(NCORES)), trace=_trace)
    results = res.results
    out = np.concatenate([np.asarray(r["out"]) for r in results], axis=0)
    if _trace:
        kernel._last = res
    return out.astype(np.float32)


# revision 30
# speedup vs baseline: 1.0100x; 1.0100x over previous
"""Trainium2 distributed kernel for nn_AutoCorrelationLayer (FourierBlock).

Reference math:
    q   = queries @ Wq.T + bq                  (B, L, E)
    xf  = rfft(q, axis=1)[:, :M, :]            keep 32 low modes
    y_m = xf_m @ (W1r_m + i W1i_m)             per-mode ExE complex mix
    x   = irfft(pad(y), n=L, axis=1)
    out = x @ Wo.T + bo

Only M=32 of 1025 frequency bins survive, so the FFTs collapse to small
DFT matmuls, and Wq / Wo can be folded into the per-mode weights on the
host:  W'_m = Wq.T @ (W1r_m + i W1i_m) @ Wo.T.  Device pipeline:

  A: qf_m  = DFT_lo(queries)                  batch-parallel (4 batches/core)
  -- AllToAll: batch-shard -> mode-shard
  B: y_m   = qf_m @ W'_m                      mode-parallel  (4 modes/core)
  -- AllToAll: mode-shard -> batch-shard
  C: out   = iDFT_lo(y)                       batch-parallel

Raw bass (manual semaphores): walrus's DIRECT2D DMA template accepts at
most one sync wait, so all DMA ordering is done with engine-level
wait_ge instructions and program order on the two HWDGE rings (SP=input
streams, ACT=weights/evictions/output).  All matmuls run in bf16 with
f32 PSUM accumulation.
"""

import sys
from contextlib import ExitStack

import numpy as np

sys.path.insert(0, "/opt/trn_rl_repo")

import concourse.bass as bass  # noqa: E402
import concourse.mybir as mybir  # noqa: E402
from concourse.bass_utils import run_bass_kernel_spmd  # noqa: E402

import ml_dtypes  # noqa: E402

BF16 = ml_dtypes.bfloat16

B, L, E, MODES = 32, 2048, 512, 32
NCORES = 8
BL = B // NCORES          # local batches per core (4)
ML = MODES // NCORES      # local modes per core (4)
NCH = E // 128            # 128-partition chunks of E (4)
KT = L // 128             # k-tiles along L (16)
GRP = 2 * ML              # cols per mode-group in DFT output (4 cos + 4 sin)

_nc_cache = None


def build_nc():
    f32 = mybir.dt.float32
    bf16 = mybir.dt.bfloat16

    nc = bass.Bass()

    q_ext = nc.declare_dram_parameter("q", [BL, L, E], bf16, isOutput=False)
    ft_ext = nc.declare_dram_parameter("ft", [128, KT * 64], bf16, isOutput=False)
    w_ext = nc.declare_dram_parameter("w", [ML, 128, 2 * NCH * E], bf16, isOutput=False)
    g_ext = nc.declare_dram_parameter("g", [128, L], bf16, isOutput=False)
    mb_ext = nc.declare_dram_parameter("mb", [B, E], f32, isOutput=False)
    out_ext = nc.declare_dram_parameter("out", [BL, L, E], bf16, isOutput=True)

    # A2A bounces. a1: [dest j][b 4][i 512][col 8]; a2: [dest j][b 4][tr 8][p 512]
    a1_in = nc.dram_tensor("a1_in", [NCORES, BL * E * GRP], bf16)
    a1_out = nc.dram_tensor("a1_out", [NCORES, BL * E * GRP], bf16)
    a2_in = nc.dram_tensor("a2_in", [NCORES, BL * GRP * E], bf16)
    a2_out = nc.dram_tensor("a2_out", [NCORES, BL * GRP * E], bf16)
    wa_in = nc.dram_tensor("wa_in", [NCORES, 16], bf16)
    wa_out = nc.dram_tensor("wa_out", [NCORES, 16], bf16)
    rg = [list(range(NCORES))]

    with ExitStack() as ctx:
        ft_sb = ctx.enter_context(nc.sbuf_tensor([128, KT * 64], bf16))
        w_sb = ctx.enter_context(nc.sbuf_tensor([128, ML * 2 * NCH * E], bf16))
        g_sb = ctx.enter_context(nc.sbuf_tensor([128, L], bf16))
        mb_sb = ctx.enter_context(nc.sbuf_tensor([B, E], f32))
        qk_sb = ctx.enter_context(nc.sbuf_tensor([128, 2 * KT * E], bf16))
        qa_sb = ctx.enter_context(nc.sbuf_tensor([128, BL * NCH * 64], bf16))
        qm_sb = ctx.enter_context(nc.sbuf_tensor([128, NCH * B * GRP], bf16))
        nq_sb = ctx.enter_context(nc.sbuf_tensor([128, NCH * B * GRP], bf16))
        ys_sb = ctx.enter_context(nc.sbuf_tensor([128, 2 * E], bf16))
        yst_sb = ctx.enter_context(nc.sbuf_tensor([128, 2 * E], bf16))
        ob_sb = ctx.enter_context(nc.sbuf_tensor([128, 2 * 16 * E], bf16))
        ps = ctx.enter_context(nc.psum_tensor([128, 4096], f32))
        (sFt, sMb, sW, sG, sMA, sEA, sS1, sCC, sQM, sNG, sMB, sAD, sEB, sS2,
         sMC, sECa, sECv) = (
            ctx.enter_context(nc.semaphore(n))
            for n in ("sFt", "sMb", "sW", "sG", "sMA", "sEA", "sS1", "sCC",
                      "sQM", "sNG", "sMB", "sAD", "sEB", "sS2", "sMC", "sECa",
                      "sECv")
        )
        sQh = [ctx.enter_context(nc.semaphore(f"sQ{i}")) for i in range(4)]
        sYs = [ctx.enter_context(nc.semaphore(f"sY{i}")) for i in range(2)]
        sOb = [ctx.enter_context(nc.semaphore(f"sO{i}")) for i in range(2)]
        sOg = [ctx.enter_context(nc.semaphore(f"sOg{i}")) for i in range(2)]
        block = ctx.enter_context(nc.Block())
        # views
        def qk_v(b, k):
            return qk_sb[:, ((b % 2) * KT + k) * E : ((b % 2) * KT + k + 1) * E]

        def ft_v(k):
            return ft_sb[:, 64 * k : 64 * (k + 1)]

        def w_v(t, j, ch):
            o = ((t * 2 + j) * NCH + ch) * E
            return w_sb[:, o : o + E]

        def psA_v(b, ch):
            bank = (b % 2) * 4 + ch
            return ps[:, 512 * bank : 512 * bank + 64]

        def psB_v(t, ri):
            x = 2 * (t % 2) + ri
            bank = 4 * (t // 2) + x
            return ps[32 * x : 32 * (x + 1), 512 * bank : 512 * (bank + 1)]

        def psC_v(idx):
            bank = idx % 8
            return ps[:, 512 * bank : 512 * (bank + 1)]

        qa_r = qa_sb.rearrange(
            "p (j b ch u) -> p j b ch u", j=NCORES, b=BL, ch=NCH, u=GRP
        )

        def qa_v(b, ch):
            return qa_r[:, :, b, ch, :]  # (128, 8, 8) strided

        qm_r = qm_sb.rearrange(
            "p (jb ch u) -> p ch u jb", jb=B, ch=NCH, u=GRP
        )
        nq_r = nq_sb.rearrange(
            "p (jb ch u) -> p ch u jb", jb=B, ch=NCH, u=GRP
        )

        def ys_v(t, ri):
            x = 2 * (t % 2) + ri
            return ys_sb[32 * x : 32 * (x + 1), (t // 2) * E : (t // 2 + 1) * E]

        def yst_v(b):
            return yst_sb[:, (b % 2) * E : ((b % 2) + 1) * E]

        def ob_v(idx):
            o = ((idx // 16) % 2) * 16 + idx % 16
            return ob_sb[:, o * E : (o + 1) * E]

        def ob_b(bb):
            o = (bb % 2) * 16
            return ob_sb[:, o * E : (o + 16) * E]

        def cdec(idx):
            return idx // 16, idx % 16  # b, l-chunk

        EV_ENG = ["a" if i % 2 == 0 else "v" for i in range(BL * 16)]

        def ev_count(eng, upto):
            return sum(1 for i in range(upto + 1) if EV_ENG[i] == eng)


        # ---------------- SP ring: input streams ----------------
        @block.sync
        def _(sync):
            for b in range(BL):
                if b >= 2:
                    sync.wait_ge(sMA, 4 * (b - 1))  # batch b-2 fully consumed
                for h in range(2):
                    sync.dma_start(
                        out=qk_sb.rearrange("p (s k e) -> p s k e", s=4, k=KT // 2)[
                            :, (b % 2) * 2 + h
                        ],
                        in_=q_ext[b].rearrange("(k p) e -> p k e", p=128)[
                            :, 8 * h : 8 * (h + 1)
                        ],
                    ).then_inc(sQh[(b % 2) * 2 + h], 16)
            # qm load (after collective 1)
            sync.wait_ge(sCC, 2)
            sync.dma_start(
                out=qm_sb.rearrange("p (j f) -> p j f", j=NCORES),
                in_=a1_out.rearrange("j (p f) -> p j f", p=128),
            ).then_inc(sQM, 16)
            # yst loads (after collective 2) interleaved with out stores
            sync.wait_ge(sCC, 3)

            def out_dmas(bb):
                for q in range(2):
                    na = ev_count("a", 16 * bb + 4 * q + 3)
                    nv = ev_count("v", 16 * bb + 4 * q + 3)
                    if na:
                        sync.wait_ge(sECa, na)
                    if nv:
                        sync.wait_ge(sECv, nv)
                    sync.dma_start(
                        out=out_ext[bb, 512 * q : 512 * (q + 1), :].rearrange(
                            "(lch p) e -> p lch e", p=128
                        ),
                        in_=ob_b(bb)[:, 4 * q * E : 4 * (q + 1) * E].rearrange(
                            "p (lch e) -> p lch e", e=E
                        ),
                    ).then_inc(sOb[bb % 2], 16)

            for b in range(BL):
                if b >= 2:
                    sync.wait_ge(sMC, 16 * (b - 1))  # batch b-2 C-matmuls done
                for half in range(2):
                    sync.dma_start(
                        out=yst_v(b)[64 * half : 64 * (half + 1), :],
                        in_=a2_out.rearrange(
                            "j (x bl pp p) -> bl j x pp p", x=4, bl=BL, pp=2, p=E
                        )[b],
                    ).then_inc(sYs[b % 2], 16)
                if b >= 1:
                    out_dmas(b - 1)
            out_dmas(BL - 1)

        # ---------------- PE: all matmuls ----------------
        @block.tensor
        def _(pe):
            pe.wait_ge(sFt, 16)  # ft loaded
            for b in range(BL):
                if b >= 2:
                    pe.wait_ge(sEA, 4 * (b - 1))  # psum bank set evicted
                for k in range(KT):
                    if k % 8 == 0:
                        pe.wait_ge(sQh[(b % 2) * 2 + k // 8], 16 * (b // 2 + 1))
                    for ch in range(NCH):
                        mm = pe.matmul(
                            psA_v(b, ch),
                            qk_v(b, k)[:, 128 * ch : 128 * (ch + 1)],
                            ft_v(k),
                            start=(k == 0),
                            stop=(k == KT - 1),
                        )
                        if k == KT - 1:
                            mm.then_inc(sMA, 1)
            # stage B
            pe.wait_ge(sW, 64)
            pe.wait_ge(sG, 16)
            pe.wait_ge(sQM, 16)
            pe.wait_ge(sNG, 1)
            for ps_i_ in range(2):
                for ch in range(NCH):
                    for tl in range(2):
                        t = 2 * ps_i_ + tl
                        lhs_r = qm_r[:, ch, t, :]
                        lhs_i = qm_r[:, ch, ML + t, :]
                        lhs_ni = nq_r[:, ch, ML + t, :]
                        first, last = ch == 0, ch == NCH - 1
                        tp0 = (0, 32 * (2 * tl + 0))
                        tp1 = (0, 32 * (2 * tl + 1))
                        pe.matmul(psB_v(t, 0), lhs_r, w_v(t, 0, ch),
                                  start=first, stop=False, tile_position=tp0)
                        pe.matmul(psB_v(t, 1), lhs_r, w_v(t, 1, ch),
                                  start=first, stop=False, tile_position=tp1)
                        m3 = pe.matmul(psB_v(t, 0), lhs_ni, w_v(t, 1, ch),
                                       start=False, stop=last, tile_position=tp0)
                        m4 = pe.matmul(psB_v(t, 1), lhs_i, w_v(t, 0, ch),
                                       start=False, stop=last, tile_position=tp1)
                        if last:
                            m3.then_inc(sMB, 1)
                            m4.then_inc(sMB, 1)
            # stage C
            for b in range(BL):
                pe.wait_ge(sYs[b % 2], 32 * (b // 2 + 1))
                for lch in range(0, 16, 2):
                    idx = b * 16 + lch
                    if idx >= 8:
                        na = ev_count("a", idx - 7)
                        nv = ev_count("v", idx - 7)
                        if na:
                            pe.wait_ge(sECa, na)
                        if nv:
                            pe.wait_ge(sECv, nv)
                    pe.matmul(
                        psC_v(idx),
                        g_sb[0:64, 128 * lch : 128 * (lch + 1)],
                        yst_v(b)[0:64, :],
                        start=True,
                        stop=True,
                        tile_position=(0, 0),
                    ).then_inc(sMC, 1)
                    pe.matmul(
                        psC_v(idx + 1),
                        g_sb[64:128, 128 * (lch + 1) : 128 * (lch + 2)],
                        yst_v(b)[64:128, :],
                        start=True,
                        stop=True,
                        tile_position=(64, 0),
                    ).then_inc(sMC, 1)

        # ------------- ACT ring: consts, evictions, staging, output -------------
        @block.scalar
        def _(act):
            act.dma_start(out=ft_sb[:], in_=ft_ext[:]).then_inc(sFt, 16)
            act.dma_start(out=mb_sb[:], in_=mb_ext[:]).then_inc(sMb, 16)
            act.dma_start(out=g_sb[:], in_=g_ext[:]).then_inc(sG, 16)
            # stage A evictions (f32 -> bf16)
            for b in range(BL):
                for ch in range(NCH):
                    act.wait_ge(sMA, 4 * b + ch + 1)
                    act.copy(
                        out=qa_v(b, ch),
                        in_=psA_v(b, ch).rearrange("p (j u) -> p j u", j=NCORES),
                    ).then_inc(sEA, 1)
            # staging 1
            act.wait_ge(sEA, 16)
            act.dma_start(
                out=a1_in.rearrange("j (p f) -> p j f", p=128),
                in_=qa_sb.rearrange("p (j f) -> p j f", j=NCORES),
            ).then_inc(sS1, 16)
            # w loads drain during collective 1
            for t in range(ML):
                act.dma_start(
                    out=w_sb[:, t * 2 * NCH * E : (t + 1) * 2 * NCH * E],
                    in_=w_ext[t],
                ).then_inc(sW, 16)
            # stage B evictions
            for t in range(ML):
                for ri in range(2):
                    if t == 0 and ri == 0:
                        act.wait_ge(sAD, 1)
                    else:
                        act.wait_ge(sMB, 4 * (t // 2) + 2 * (t % 2) + ri + 1)
                    act.copy(out=ys_v(t, ri), in_=psB_v(t, ri)).then_inc(sEB, 1)
            # staging 2 (per col-group as its evictions finish)
            for x in range(4):
                act.wait_ge(sEB, 5 + x)
                act.dma_start(
                    out=a2_in.rearrange("j (x blc) -> x j blc", x=4, blc=BL * 2 * E)[x],
                    in_=ys_sb[32 * x : 32 * (x + 1), :],
                ).then_inc(sS2, 16)
            # stage C evictions (ACT share)
            for idx in range(BL * 16):
                if EV_ENG[idx] != "a":
                    continue
                act.wait_ge(sMC, idx + 1)
                bb = idx // 16
                if bb >= 2 and idx % 16 == 0:
                    act.wait_ge(sOb[bb % 2], 32 * ((bb - 2) // 2 + 1))
                    act.wait_ge(sOg[bb % 2], 32 * ((bb - 2) // 2 + 1))
                act.copy(out=ob_v(idx), in_=psC_v(idx)).then_inc(sECa, 1)

        # ------------- DVE: negation, bias add, half the C evictions -------------
        @block.vector
        def _(dve):
            dve.wait_ge(sQM, 16)
            dve.tensor_scalar_mul(nq_sb[:], qm_sb[:], -1.0).then_inc(sNG, 1)
            dve.wait_ge(sMb, 16)  # mb loaded
            dve.wait_ge(sMB, 1)   # t=0 yr chain done
            dve.tensor_add(psB_v(0, 0), psB_v(0, 0), mb_sb[:]).then_inc(sAD, 1)
            first_v = min(i for i in range(16) if EV_ENG[i] == "v")
            for idx in range(BL * 16):
                if EV_ENG[idx] != "v":
                    continue
                dve.wait_ge(sMC, idx + 1)
                bb = idx // 16
                if bb >= 2 and idx % 16 == first_v:
                    dve.wait_ge(sOb[bb % 2], 32 * ((bb - 2) // 2 + 1))
                    dve.wait_ge(sOg[bb % 2], 32 * ((bb - 2) // 2 + 1))
                dve.tensor_copy(ob_v(idx), psC_v(idx)).then_inc(sECv, 1)

        # ---------------- GPSIMD: collectives ----------------
        @block.gpsimd
        def _(gp):
            gp.collective_compute(
                "AllToAll",
                mybir.AluOpType.bypass,
                replica_groups=rg,
                ins=[wa_in[:]],
                outs=[wa_out[:]],
            ).then_inc(sCC, 1)
            gp.wait_ge(sS1, 16)
            gp.collective_compute(
                "AllToAll",
                mybir.AluOpType.bypass,
                replica_groups=rg,
                ins=[a1_in[:]],
                outs=[a1_out[:]],
            ).then_inc(sCC, 1)
            gp.wait_ge(sS2, 64)
            gp.collective_compute(
                "AllToAll",
                mybir.AluOpType.bypass,
                replica_groups=rg,
                ins=[a2_in[:]],
                outs=[a2_out[:]],
            ).then_inc(sCC, 1)
            for bb in range(BL):
                for q in range(2, 4):
                    gp.wait_ge(sECa, ev_count("a", 16 * bb + 4 * q + 3))
                    gp.wait_ge(sECv, ev_count("v", 16 * bb + 4 * q + 3))
                    gp.dma_start(
                        out=out_ext[bb, 512 * q : 512 * (q + 1), :].rearrange(
                            "(lch p) e -> p lch e", p=128
                        ),
                        in_=ob_b(bb)[:, 4 * q * E : 4 * (q + 1) * E].rearrange(
                            "p (lch e) -> p lch e", e=E
                        ),
                    ).then_inc(sOg[bb % 2], 16)

    return nc


def _host_prep(queries, Wq, bq, W1r, W1i, Wo, bo):
    """Fold Wq/Wo into per-mode weights, build DFT matrices, shard per core."""
    l = np.arange(L)
    m = np.arange(MODES)
    ang = 2.0 * np.pi * np.outer(m, l) / L          # (M, L)
    cos_ml = np.cos(ang)
    sin_ml = np.sin(ang)

    # DFT moving tiles, packed [p_in_tile, k*64 + c]; c: group g -> [cos, -sin]
    ft = np.empty((L, 64), np.float32)
    for g in range(NCORES):
        ft[:, GRP * g : GRP * g + ML] = cos_ml[4 * g : 4 * g + ML].T
        ft[:, GRP * g + ML : GRP * (g + 1)] = -sin_ml[4 * g : 4 * g + ML].T
    ft = np.ascontiguousarray(
        ft.reshape(KT, 128, 64).transpose(1, 0, 2).reshape(128, KT * 64)
    )

    # Folded mode weights: W'_m = Wq.T @ (W1r_m + i W1i_m) @ Wo.T
    Wq64 = Wq.astype(np.float64)
    Wo64 = Wo.astype(np.float64)
    Wpr = np.empty((E, E, MODES), np.float32)
    Wpi = np.empty((E, E, MODES), np.float32)
    for mm in range(MODES):
        ar = Wq64.T @ W1r[:, :, mm].astype(np.float64)
        ai = Wq64.T @ W1i[:, :, mm].astype(np.float64)
        Wpr[:, :, mm] = (ar @ Wo64.T).astype(np.float32)
        Wpi[:, :, mm] = (ai @ Wo64.T).astype(np.float32)

    # Inverse DFT rows g[k = j*8 + t*2 + ri, l]
    cm = np.where(m == 0, 1.0, 2.0)
    g_mat = np.empty((64, L), np.float32)
    for r in range(64):
        c, x, pp = r // 8, (r % 8) // 2, r % 2
        tl, ri = x // 2, x % 2
        mm = 4 * c + 2 * pp + tl
        if ri == 0:
            g_mat[r] = cm[mm] * cos_ml[mm] / L
        else:
            g_mat[r] = -cm[mm] * sin_ml[mm] / L
            if mm == 0:
                g_mat[r] = 0.0  # irfft ignores Im(bin 0)

    out_bias = (
        bo.astype(np.float64)
        + bq.astype(np.float64) @ W1r[:, :, 0].astype(np.float64) @ Wo64.T
    ).astype(np.float32)

    ft_b = ft.astype(BF16)
    g_b = np.vstack([g_mat, g_mat]).astype(BF16)

    in_maps = []
    for c in range(NCORES):
        w_pack = np.empty((ML, 128, 2, NCH, E), np.float32)
        for t in range(ML):
            mm = 4 * c + t
            for ch in range(NCH):
                w_pack[t, :, 0, ch] = Wpr[128 * ch : 128 * (ch + 1), :, mm]
                w_pack[t, :, 1, ch] = Wpi[128 * ch : 128 * (ch + 1), :, mm]
        w_pack = w_pack.reshape(ML, 128, 2 * NCH * E)
        in_maps.append(
            {
                "q": np.ascontiguousarray(queries[BL * c : BL * (c + 1)]).astype(BF16),
                "ft": ft_b,
                "w": w_pack.astype(BF16),
                "g": g_b,
                "mb": np.broadcast_to(
                    L * out_bias[None, :] if c == 0 else np.zeros((1, E), np.float32),
                    (B, E),
                ).astype(np.float32),
            }
        )
    return in_maps


def kernel(queries, Wq, bq, W1r, W1i, Wo, bo, _trace=False):
    global _nc_cache
    if _nc_cache is None:
        _nc_cache = build_nc()
    nc = _nc_cache

    in_maps = _host_prep(queries, Wq, bq, W1r, W1i, Wo, bo)
    res = run_bass_kernel_spmd(nc, in_maps, core_ids=list(range(NCORES)), trace=_trace)
    results = res.results
    out = np.concatenate([np.asarray(r["out"]) for r in results], axis=0)
    if _trace:
        kernel._last = res
    return out.astype(np.float32)
